# revision 12
# baseline (speedup 1.0000x reference)
"""AFT-Full forward on 8 Trainium2 NeuronCores (Bass/Tile, SPMD).

Reference (per batch b):
    Q = x^T wq^T + bq ; K = x^T wk^T + bk ; V = x^T wv^T + bv      # [T, H]
    ew = exp(wbias[:T, :T])                                        # [T, T]
    num = ew @ (exp(K) * V) ; den = ew @ exp(K)                    # [T, H]
    out = (sigmoid(Q) * num / den) @ wp^T + bp                     # [T, DIM]

Sharding (2-D): 2 batch-groups x 4 query-time slices.
Core c -> (h = c // 4 batch-group, g = c % 4 t-slice of 1024 rows).
 - core c computes K/V (and Z = [exp(K)*V | exp(K)]) for its OWN batch
   b = 4h + g over the full sequence; two 4-rank AllGathers (one per
   half of the sequence, so compute overlaps the second gather) share Z
   with the batch-group.
 - core c then produces out[4h:4h+4, g*1024:(g+1)*1024, :] using only
   wbias rows [g*1024, (g+1)*1024)  (host pre-transposes them to [s, t]).

Z travels partition-major ([128, s*128+h] rather than [s, h]) so every
DMA touching it moves contiguous multi-KB rows; the Z writes go through
SWDGE (gpsimd) so they are not queued behind the big HWDGE streams.

All matmul operands are bf16 (PE runs fp32 matmul at 1/4 rate); all
accumulation is fp32 in PSUM; exp/divide run in fp32. Sigmoid is
computed as 1/(1+exp(-Q-bq)) on the Exp LUT so the scalar engine never
reloads activation tables. Measured end-to-end relative error vs the
fp32 reference ~4.5e-3.

Biases: bq is folded into the exp activation (host passes -bq); bp is
folded into the output projection via an appended ones-row; bk/bv are
folded into the K/V matmul as a rank-1 (K=1) accumulation step.
"""

import numpy as np
import ml_dtypes

B, DIM, T, H = 8, 512, 4096, 64
H2 = 2 * H
NCORES = 8
NG = 2            # batch groups
G = 4             # t-slices per group (ranks per collective group)
NB = B // NG      # batches per group = 4
TSL = T // G      # 1024 t rows per core
SCH = T // 128    # 32 s-chunks
SPLITS = [6, 9, 11, 6]   # pipelined AllGather split sizes (s-chunks)
SOFF = [0]
for _n in SPLITS:
    SOFF.append(SOFF[-1] + _n)
NSP = len(SPLITS)
DCH = DIM // 128  # 4 contraction chunks for projections

BF16 = ml_dtypes.bfloat16

_CACHE = {}
RUN_KWARGS = {}        # test harness may set {"trace": True}
LAST_RESULT = [None]   # test harness reads exec_time_ns off this


def _build():
    import concourse.mybir as mybir
    import concourse.tile as tile
    from concourse import bacc

    fp32 = mybir.dt.float32
    bf16 = mybir.dt.bfloat16
    AF = mybir.ActivationFunctionType

    nc = bacc.Bacc("TRN2", target_bir_lowering=False, debug=False,
                   num_devices=NCORES)

    xb_ext = nc.dram_tensor("xb", [DIM, T], bf16, kind="ExternalInput").ap()
    xq_ext = nc.dram_tensor("xq", [NB, DIM, TSL], bf16, kind="ExternalInput").ap()
    ewb_ext = nc.dram_tensor("ewb", [T, TSL], bf16, kind="ExternalInput").ap()
    wkv_ext = nc.dram_tensor("wkv", [DIM, H2], bf16, kind="ExternalInput").ap()
    wqt_ext = nc.dram_tensor("wqt", [DIM, H], bf16, kind="ExternalInput").ap()
    wpta_ext = nc.dram_tensor("wpta", [H + 1, DIM], bf16, kind="ExternalInput").ap()
    bq_ext = nc.dram_tensor("bq", [H, 1], fp32, kind="ExternalInput").ap()
    bkv_ext = nc.dram_tensor("bkv", [1, H2], bf16, kind="ExternalInput").ap()
    out_ext = nc.dram_tensor("out", [NB, TSL, DIM], fp32, kind="ExternalOutput").ap()

    groups = [list(range(0, G)), list(range(G, 2 * G))]

    with tile.TileContext(nc) as tc:
        with (
            tc.tile_pool(name="const", bufs=1) as cpool,
            tc.tile_pool(name="res", bufs=1) as rpool,
            tc.tile_pool(name="work", bufs=3) as wpool,
            tc.tile_pool(name="dram", bufs=1, space="DRAM") as dpool,
        ):
            # ---- constants ----
            wkv_sb = cpool.tile([128, DCH, H2], bf16)
            nc.sync.dma_start(wkv_sb[:], wkv_ext.rearrange("(n p) m -> p n m", p=128))
            wqt_sb = cpool.tile([128, DCH, H], bf16)
            nc.sync.dma_start(wqt_sb[:], wqt_ext.rearrange("(n p) m -> p n m", p=128))
            wpta_sb = cpool.tile([H + 1, DIM], bf16)
            nc.sync.dma_start(wpta_sb[:], wpta_ext[:])
            bq_sb = cpool.tile([H, 1], fp32)
            nc.sync.dma_start(bq_sb[:], bq_ext[:])
            bkv_sb = cpool.tile([1, H2], bf16)
            nc.sync.dma_start(bkv_sb[:], bkv_ext[:])
            ones_sb = cpool.tile([1, 128], bf16)
            nc.vector.memset(ones_sb[:], 1.0)

            # ---- resident tensors; big HWDGE streams issue up front ----
            xb_sb = rpool.tile([128, DCH, T], bf16)       # 8KB/part
            for d in range(DCH):
                nc.sync.dma_start(xb_sb[:, d, :],
                                  xb_ext[d * 128:(d + 1) * 128, :])
            xq_sbs = [wpool.tile([128, DCH, TSL], bf16, tag="xq", bufs=2,
                                 name=f"xq_sb{bl}") for bl in range(NB)]
            for bl in range(2):   # bl2/3 stream later (slot rotation)
                nc.sync.dma_start(
                    xq_sbs[bl][:],
                    xq_ext[bl].rearrange("(n p) t -> p n t", p=128))
            ew_sb = rpool.tile([128, SCH, TSL], bf16)     # exp(wbias^T), 64KB/part
            for s in range(SCH):
                nc.sync.dma_start(ew_sb[:, s, :],
                                  ewb_ext[s * 128:(s + 1) * 128, :])
            z_acc = rpool.tile([128, SCH, H2], bf16)      # own-batch Z, p-major
            z_res = [rpool.tile([128, NB, SPLITS[k], H2], bf16,
                                name=f"z_res{k}")
                     for k in range(NSP)]                 # gathered Z, 32KB/part
            sq_sb = rpool.tile([H, NB, TSL], fp32)        # sigmoid(Q^T)

            z_own = [dpool.tile([128, SPLITS[i] * H2], bf16, name=f"z_own{i}")
                     for i in range(NSP)]
            z_all = [dpool.tile([G * 128, SPLITS[i] * H2], bf16,
                                name=f"z_all{i}")
                     for i in range(NSP)]

            # ---- phase 1: Z for own batch; AllGather per half ----
            with tc.tile_pool(name="ps_a", bufs=2, space="PSUM") as ps_a:
                for k in range(NSP):
                    for sl in range(SPLITS[k]):
                        s = SOFF[k] + sl
                        kv_ps = ps_a.tile([128, H2], fp32, tag="kv", bufs=4)
                        for d in range(DCH):
                            nc.tensor.matmul(
                                kv_ps[:], xb_sb[:, d, s * 128:(s + 1) * 128],
                                wkv_sb[:, d, :], start=(d == 0), stop=False)
                        # rank-1 bias fold: += ones^T @ [bv | bk]
                        nc.tensor.matmul(kv_ps[:], ones_sb[:], bkv_sb[:],
                                         start=False, stop=True)
                        ek_sb = wpool.tile([128, H], fp32, tag="ek")
                        nc.scalar.activation(ek_sb[:], kv_ps[:, H:H2], AF.Exp)
                        nc.vector.tensor_mul(z_acc[:, s, 0:H], kv_ps[:, 0:H],
                                             ek_sb[:])
                        nc.vector.tensor_copy(z_acc[:, s, H:H2], ek_sb[:])
                    # SWDGE so this is not queued behind the HWDGE streams
                    nc.gpsimd.dma_start(
                        z_own[k][:],
                        z_acc[:, SOFF[k]:SOFF[k + 1], :])
                    nc.gpsimd.collective_compute(
                        "AllGather", mybir.AluOpType.bypass,
                        replica_groups=groups,
                        ins=[z_own[k].opt()], outs=[z_all[k].opt()],
                    )

                # ---- phase 2a: sigmoid(Q) (PE idles during the gathers) ----
                for bl in range(NB):
                    xq_sb = xq_sbs[bl]
                    if bl >= 2:
                        nc.sync.dma_start(
                            xq_sb[:],
                            xq_ext[bl].rearrange("(n p) t -> p n t", p=128),
                        )
                    for th in range(TSL // 512):
                        q_ps = ps_a.tile([H, 512], fp32, tag="q")
                        for d in range(DCH):
                            nc.tensor.matmul(
                                q_ps[:], wqt_sb[:, d, :],
                                xq_sb[:, d, th * 512:(th + 1) * 512],
                                start=(d == 0), stop=(d == DCH - 1))
                        # sigmoid via the Exp LUT (avoids table reloads):
                        # sq = 1 / (1 + exp(-Q - bq));  bq_ext holds -bq.
                        eq_sb = wpool.tile([H, 512], fp32, tag="eq")
                        nc.scalar.activation(eq_sb[:], q_ps[:], AF.Exp,
                                             bias=bq_sb[:], scale=-1.0)
                        nc.vector.tensor_scalar_add(eq_sb[:], eq_sb[:], 1.0)
                        nc.vector.reciprocal_approx_fast(
                            sq_sb[:, bl, th * 512:(th + 1) * 512], eq_sb[:])

                # ---- phase 2b: ew = exp(wbias^T) in place (after the
                # sigmoid exps in ACT order, so the Q chain finishes early
                # and releases its PSUM banks before the main matmuls) ----
                for s in range(SCH):
                    nc.scalar.activation(ew_sb[:, s, :], ew_sb[:, s, :], AF.Exp)

                # ---- gathered Z -> SBUF resident (per split, per batch) ----
                for k in range(NSP):
                    for bl in range(NB):
                        nc.sync.dma_start(
                            z_res[k][:, bl, :, :],
                            z_all[k][bl * 128:(bl + 1) * 128, :]
                            .rearrange("p (s h) -> p s h", s=SPLITS[k]),
                        )

            # ---- phase 3: num/den matmuls + epilogue ----
            # Single PSUM tag: 4 live [128,1024] accumulators (8 banks);
            # the oproj tiles reuse the slots as the accumulators retire.
            def epilogue_dve(nd_ps, bl, yt_sb):
                # reciprocal_approx_* are custom DVE ops — feed them
                # from SBUF, not PSUM (PSUM reads gave garbage).
                den_sb = wpool.tile([H, TSL], fp32, tag="den", bufs=1,
                                    name=f"den{bl}")
                nc.vector.tensor_copy(den_sb[:], nd_ps[H:H2, :])
                rcp_sb = wpool.tile([H, TSL], fp32, tag="rcp", bufs=1,
                                    name=f"rcp{bl}")
                nc.vector.reciprocal_approx_fast(rcp_sb[:], den_sb[:])
                nc.vector.tensor_mul(yt_sb[0:H, :], nd_ps[0:H, :], rcp_sb[:])
                nc.vector.tensor_mul(yt_sb[0:H, :], yt_sb[0:H, :],
                                     sq_sb[:, bl, :])
                nc.vector.memset(yt_sb[H:H + 1, :], 1.0)

            def oproj(bl, yt_sb, ps_b):
                for tch in range(TSL // 128):
                    o_ps = ps_b.tile([128, DIM], fp32, tag="mn", bufs=4,
                                     name=f"o_ps{bl}_{tch}")
                    nc.tensor.matmul(
                        o_ps[:], yt_sb[:, tch * 128:(tch + 1) * 128],
                        wpta_sb[:], start=True, stop=True)
                    o_sb = wpool.tile([128, DIM], fp32, tag="o", bufs=4,
                                      name=f"o_sb{bl}_{tch}")
                    if tch % 2 == 0:
                        nc.vector.tensor_copy(o_sb[:], o_ps[:])
                    else:
                        nc.scalar.copy(o_sb[:], o_ps[:])
                    nc.sync.dma_start(
                        out_ext[bl, tch * 128:(tch + 1) * 128, :], o_sb[:])

            with tc.tile_pool(name="ps_b", bufs=1, space="PSUM") as ps_b:
                nd_pss = [ps_b.tile([128, TSL], fp32, tag="mn", bufs=4,
                                    name=f"nd_ps{bl}")
                          for bl in range(NB)]

                def mm(bl, s):
                    k = next(i for i in range(NSP) if SOFF[i + 1] > s)
                    sl = s - SOFF[k]
                    for th in range(TSL // 512):
                        nc.tensor.matmul(
                            nd_pss[bl][:, th * 512:(th + 1) * 512],
                            z_res[k][:, bl, sl, :],
                            ew_sb[:, s, th * 512:(th + 1) * 512],
                            start=(s == 0), stop=(s == SCH - 1))

                # splits 0..NSP-2: all batches (overlap the later gathers)
                for s in range(SOFF[NSP - 1]):
                    for bl in range(NB):
                        mm(bl, s)
                # last split: stagger per batch so epilogues pipeline
                yt_sbs = [None] * NB
                for bl in range(NB):
                    for s in range(SOFF[NSP - 1], SCH):
                        mm(bl, s)
                    yt_sbs[bl] = wpool.tile([H + 1, TSL], bf16, tag="yt",
                                            bufs=2, name=f"yt{bl}")
                    epilogue_dve(nd_pss[bl], bl, yt_sbs[bl])
                    if bl >= 1:
                        oproj(bl - 1, yt_sbs[bl - 1], ps_b)
                oproj(NB - 1, yt_sbs[NB - 1], ps_b)

    nc.compile()
    return nc


def _get_nc():
    if "nc" not in _CACHE:
        _CACHE["nc"] = _build()
    return _CACHE["nc"]


def kernel(x, wq, bq, wk, bk, wv, bv, wp, bp, wbias):
    from concourse.bass_utils import run_bass_kernel_spmd

    x = np.asarray(x); wbias = np.asarray(wbias)
    wkv = np.concatenate([np.asarray(wv).T, np.asarray(wk).T], axis=1).astype(BF16)
    bkv = np.concatenate([np.asarray(bv), np.asarray(bk)])[None, :].astype(BF16)
    wqt = np.asarray(wq).T.astype(BF16)
    wpta = np.concatenate([np.asarray(wp).T, np.asarray(bp)[None, :]],
                          axis=0).astype(BF16)
    bq_in = (-np.asarray(bq)).reshape(H, 1).astype(np.float32)

    in_maps = []
    for c in range(NCORES):
        g, h = c % G, c // G
        tsl = slice(g * TSL, (g + 1) * TSL)
        in_maps.append({
            "xb": x[G * h + g].astype(BF16),
            "xq": x[NB * h:NB * (h + 1), :, tsl].astype(BF16),
            "ewb": np.ascontiguousarray(wbias[tsl, :].T).astype(BF16),
            "wkv": wkv, "wqt": wqt, "wpta": wpta, "bq": bq_in, "bkv": bkv,
        })

    nc = _get_nc()
    res = run_bass_kernel_spmd(nc, in_maps, core_ids=list(range(NCORES)),
                               **RUN_KWARGS)
    LAST_RESULT[0] = res

    out_full = np.empty((B, T, DIM), np.float32)
    for c in range(NCORES):
        g, h = c % G, c // G
        out_full[NB * h:NB * (h + 1), g * TSL:(g + 1) * TSL, :] = \
            res.results[c]["out"]
    return (out_full, out_full)


# revision 13
# speedup vs baseline: 1.0889x; 1.0889x over previous
"""AFT-Full forward on 8 Trainium2 NeuronCores (Bass/Tile, SPMD).

Reference (per batch b):
    Q = x^T wq^T + bq ; K = x^T wk^T + bk ; V = x^T wv^T + bv      # [T, H]
    ew = exp(wbias[:T, :T])                                        # [T, T]
    num = ew @ (exp(K) * V) ; den = ew @ exp(K)                    # [T, H]
    out = (sigmoid(Q) * num / den) @ wp^T + bp                     # [T, DIM]

Sharding (2-D): 2 batch-groups x 4 query-time slices.
Core c -> (h = c // 4 batch-group, g = c % 4 t-slice of 1024 rows).
 - core c computes K/V (and Z = [exp(K)*V | exp(K)]) for its OWN batch
   b = 4h + g over the full sequence; two 4-rank AllGathers (one per
   half of the sequence, so compute overlaps the second gather) share Z
   with the batch-group.
 - core c then produces out[4h:4h+4, g*1024:(g+1)*1024, :] using only
   wbias rows [g*1024, (g+1)*1024)  (host pre-transposes them to [s, t]).

Z travels partition-major ([128, s*128+h] rather than [s, h]) so every
DMA touching it moves contiguous multi-KB rows; the Z writes go through
SWDGE (gpsimd) so they are not queued behind the big HWDGE streams.

All matmul operands are bf16 (PE runs fp32 matmul at 1/4 rate); all
accumulation is fp32 in PSUM; exp/divide run in fp32. Sigmoid is
computed as 1/(1+exp(-Q-bq)) on the Exp LUT so the scalar engine never
reloads activation tables. Measured end-to-end relative error vs the
fp32 reference ~4.5e-3.

Biases: bq is folded into the exp activation (host passes -bq); bp is
folded into the output projection via an appended ones-row; bk/bv are
folded into the K/V matmul as a rank-1 (K=1) accumulation step.
"""

import numpy as np
import ml_dtypes

B, DIM, T, H = 8, 512, 4096, 64
H2 = 2 * H
NCORES = 8
NG = 2            # batch groups
G = 4             # t-slices per group (ranks per collective group)
NB = B // NG      # batches per group = 4
TSL = T // G      # 1024 t rows per core
SCH = T // 128    # 32 s-chunks
SPLITS = [10, 12, 10]    # pipelined AllGather split sizes (s-chunks)
SOFF = [0]
for _n in SPLITS:
    SOFF.append(SOFF[-1] + _n)
NSP = len(SPLITS)
DCH = DIM // 128  # 4 contraction chunks for projections

BF16 = ml_dtypes.bfloat16

_CACHE = {}
RUN_KWARGS = {}        # test harness may set {"trace": True}
LAST_RESULT = [None]   # test harness reads exec_time_ns off this


def _build():
    import concourse.mybir as mybir
    import concourse.tile as tile
    from concourse import bacc

    fp32 = mybir.dt.float32
    bf16 = mybir.dt.bfloat16
    AF = mybir.ActivationFunctionType

    nc = bacc.Bacc("TRN2", target_bir_lowering=False, debug=False,
                   num_devices=NCORES)

    xb_ext = nc.dram_tensor("xb", [DIM, T], bf16, kind="ExternalInput").ap()
    xq_ext = nc.dram_tensor("xq", [NB, DIM, TSL], bf16, kind="ExternalInput").ap()
    ewb_ext = nc.dram_tensor("ewb", [T, TSL], bf16, kind="ExternalInput").ap()
    wkv_ext = nc.dram_tensor("wkv", [DIM, H2], bf16, kind="ExternalInput").ap()
    wqt_ext = nc.dram_tensor("wqt", [DIM, H], bf16, kind="ExternalInput").ap()
    wpta_ext = nc.dram_tensor("wpta", [H + 1, DIM], bf16, kind="ExternalInput").ap()
    bq_ext = nc.dram_tensor("bq", [H, 1], fp32, kind="ExternalInput").ap()
    bkv_ext = nc.dram_tensor("bkv", [1, H2], bf16, kind="ExternalInput").ap()
    out_ext = nc.dram_tensor("out", [NB, TSL, DIM], fp32, kind="ExternalOutput").ap()

    groups = [list(range(0, G)), list(range(G, 2 * G))]

    with tile.TileContext(nc) as tc:
        with (
            tc.tile_pool(name="const", bufs=1) as cpool,
            tc.tile_pool(name="res", bufs=1) as rpool,
            tc.tile_pool(name="work", bufs=3) as wpool,
            tc.tile_pool(name="dram", bufs=1, space="DRAM") as dpool,
        ):
            # ---- constants ----
            wkv_sb = cpool.tile([128, DCH, H2], bf16)
            nc.sync.dma_start(wkv_sb[:], wkv_ext.rearrange("(n p) m -> p n m", p=128))
            wqt_sb = cpool.tile([128, DCH, H], bf16)
            nc.sync.dma_start(wqt_sb[:], wqt_ext.rearrange("(n p) m -> p n m", p=128))
            wpta_sb = cpool.tile([H + 1, DIM], bf16)
            nc.sync.dma_start(wpta_sb[:], wpta_ext[:])
            bq_sb = cpool.tile([H, 1], fp32)
            nc.sync.dma_start(bq_sb[:], bq_ext[:])
            bkv_sb = cpool.tile([1, H2], bf16)
            nc.sync.dma_start(bkv_sb[:], bkv_ext[:])
            ones_sb = cpool.tile([1, 128], bf16)
            nc.vector.memset(ones_sb[:], 1.0)

            # ---- resident tensors; big HWDGE streams issue up front ----
            xb_sb = rpool.tile([128, DCH, T], bf16)       # 8KB/part
            for d in range(DCH):
                nc.sync.dma_start(xb_sb[:, d, :],
                                  xb_ext[d * 128:(d + 1) * 128, :])
            xq_sbs = [wpool.tile([128, DCH, TSL], bf16, tag="xq", bufs=2,
                                 name=f"xq_sb{bl}") for bl in range(NB)]
            for bl in range(2):   # bl2/3 stream later (slot rotation)
                nc.sync.dma_start(
                    xq_sbs[bl][:],
                    xq_ext[bl].rearrange("(n p) t -> p n t", p=128))
            ew_sb = rpool.tile([128, SCH, TSL], bf16)     # exp(wbias^T), 64KB/part
            for s in range(SCH):
                nc.sync.dma_start(ew_sb[:, s, :],
                                  ewb_ext[s * 128:(s + 1) * 128, :])
            z_acc = rpool.tile([128, SCH, H2], bf16)      # own-batch Z, p-major
            z_res = [rpool.tile([128, NB, SPLITS[k], H2], bf16,
                                name=f"z_res{k}")
                     for k in range(NSP)]                 # gathered Z, 32KB/part
            sq_sb = rpool.tile([H, NB, TSL], fp32)        # sigmoid(Q^T)

            z_own = [dpool.tile([128, SPLITS[i] * H2], bf16, name=f"z_own{i}")
                     for i in range(NSP)]
            z_all = [dpool.tile([G * 128, SPLITS[i] * H2], bf16,
                                name=f"z_all{i}")
                     for i in range(NSP)]

            # ---- phase 1: Z for own batch; AllGather per half ----
            with tc.tile_pool(name="ps_a", bufs=2, space="PSUM") as ps_a:
                for k in range(NSP):
                    for sl in range(SPLITS[k]):
                        s = SOFF[k] + sl
                        kv_ps = ps_a.tile([128, H2], fp32, tag="kv", bufs=4)
                        for d in range(DCH):
                            nc.tensor.matmul(
                                kv_ps[:], xb_sb[:, d, s * 128:(s + 1) * 128],
                                wkv_sb[:, d, :], start=(d == 0), stop=False)
                        # rank-1 bias fold: += ones^T @ [bv | bk]
                        nc.tensor.matmul(kv_ps[:], ones_sb[:], bkv_sb[:],
                                         start=False, stop=True)
                        ek_sb = wpool.tile([128, H], fp32, tag="ek")
                        nc.scalar.activation(ek_sb[:], kv_ps[:, H:H2], AF.Exp)
                        nc.vector.tensor_mul(z_acc[:, s, 0:H], kv_ps[:, 0:H],
                                             ek_sb[:])
                        nc.vector.tensor_copy(z_acc[:, s, H:H2], ek_sb[:])
                    # SWDGE so this is not queued behind the HWDGE streams
                    nc.gpsimd.dma_start(
                        z_own[k][:],
                        z_acc[:, SOFF[k]:SOFF[k + 1], :])
                    nc.gpsimd.collective_compute(
                        "AllGather", mybir.AluOpType.bypass,
                        replica_groups=groups,
                        ins=[z_own[k].opt()], outs=[z_all[k].opt()],
                    )

                # ---- phase 2a: sigmoid(Q) (PE idles during the gathers) ----
                for bl in range(NB):
                    xq_sb = xq_sbs[bl]
                    if bl >= 2:
                        nc.sync.dma_start(
                            xq_sb[:],
                            xq_ext[bl].rearrange("(n p) t -> p n t", p=128),
                        )
                    for th in range(TSL // 512):
                        q_ps = ps_a.tile([H, 512], fp32, tag="q")
                        for d in range(DCH):
                            nc.tensor.matmul(
                                q_ps[:], wqt_sb[:, d, :],
                                xq_sb[:, d, th * 512:(th + 1) * 512],
                                start=(d == 0), stop=(d == DCH - 1))
                        # sigmoid via the Exp LUT (avoids table reloads):
                        # sq = 1 / (1 + exp(-Q - bq));  bq_ext holds -bq.
                        eq_sb = wpool.tile([H, 512], fp32, tag="eq")
                        nc.scalar.activation(eq_sb[:], q_ps[:], AF.Exp,
                                             bias=bq_sb[:], scale=-1.0)
                        nc.vector.tensor_scalar_add(eq_sb[:], eq_sb[:], 1.0)
                        nc.vector.reciprocal_approx_fast(
                            sq_sb[:, bl, th * 512:(th + 1) * 512], eq_sb[:])

                # ---- phase 2b: ew = exp(wbias^T) in place (after the
                # sigmoid exps in ACT order, so the Q chain finishes early
                # and releases its PSUM banks before the main matmuls) ----
                for s in range(SCH):
                    nc.scalar.activation(ew_sb[:, s, :], ew_sb[:, s, :], AF.Exp)

                # ---- gathered Z -> SBUF resident (per split, per batch) ----
                for k in range(NSP):
                    for bl in range(NB):
                        nc.sync.dma_start(
                            z_res[k][:, bl, :, :],
                            z_all[k][bl * 128:(bl + 1) * 128, :]
                            .rearrange("p (s h) -> p s h", s=SPLITS[k]),
                        )

            # ---- phase 3: num/den matmuls + epilogue ----
            # Single PSUM tag: 4 live [128,1024] accumulators (8 banks);
            # the oproj tiles reuse the slots as the accumulators retire.
            def epilogue_dve(nd_ps, bl, yt_sb):
                # reciprocal_approx_* are custom DVE ops — feed them
                # from SBUF, not PSUM (PSUM reads gave garbage).
                den_sb = wpool.tile([H, TSL], fp32, tag="den", bufs=1,
                                    name=f"den{bl}")
                nc.scalar.copy(den_sb[:], nd_ps[H:H2, :])
                rcp_sb = wpool.tile([H, TSL], fp32, tag="rcp", bufs=1,
                                    name=f"rcp{bl}")
                nc.vector.reciprocal_approx_fast(rcp_sb[:], den_sb[:])
                nc.vector.tensor_mul(yt_sb[0:H, :], nd_ps[0:H, :], rcp_sb[:])
                nc.vector.tensor_mul(yt_sb[0:H, :], yt_sb[0:H, :],
                                     sq_sb[:, bl, :])
                nc.vector.memset(yt_sb[H:H + 1, :], 1.0)

            def oproj(bl, yt_sb, ps_b):
                for tch in range(TSL // 128):
                    o_ps = ps_b.tile([128, DIM], fp32, tag="mn", bufs=4,
                                     name=f"o_ps{bl}_{tch}")
                    nc.tensor.matmul(
                        o_ps[:], yt_sb[:, tch * 128:(tch + 1) * 128],
                        wpta_sb[:], start=True, stop=True)
                    o_sb = wpool.tile([128, DIM], fp32, tag="o", bufs=4,
                                      name=f"o_sb{bl}_{tch}")
                    if tch % 4 == 0:
                        nc.vector.tensor_copy(o_sb[:], o_ps[:])
                    else:
                        nc.scalar.copy(o_sb[:], o_ps[:])
                    nc.sync.dma_start(
                        out_ext[bl, tch * 128:(tch + 1) * 128, :], o_sb[:])

            with tc.tile_pool(name="ps_b", bufs=1, space="PSUM") as ps_b:
                nd_pss = [ps_b.tile([128, TSL], fp32, tag="mn", bufs=4,
                                    name=f"nd_ps{bl}")
                          for bl in range(NB)]

                def mm(bl, s):
                    k = next(i for i in range(NSP) if SOFF[i + 1] > s)
                    sl = s - SOFF[k]
                    for th in range(TSL // 512):
                        nc.tensor.matmul(
                            nd_pss[bl][:, th * 512:(th + 1) * 512],
                            z_res[k][:, bl, sl, :],
                            ew_sb[:, s, th * 512:(th + 1) * 512],
                            start=(s == 0), stop=(s == SCH - 1))

                # splits 0..NSP-2: all batches (overlap the later gathers)
                for s in range(SOFF[NSP - 1]):
                    for bl in range(NB):
                        mm(bl, s)
                # last split: stagger per batch so epilogues pipeline
                yt_sbs = [None] * NB
                for bl in range(NB):
                    for s in range(SOFF[NSP - 1], SCH):
                        mm(bl, s)
                    yt_sbs[bl] = wpool.tile([H + 1, TSL], bf16, tag="yt",
                                            bufs=2, name=f"yt{bl}")
                    epilogue_dve(nd_pss[bl], bl, yt_sbs[bl])
                    if bl >= 1:
                        oproj(bl - 1, yt_sbs[bl - 1], ps_b)
                oproj(NB - 1, yt_sbs[NB - 1], ps_b)

    nc.compile()
    return nc


def _get_nc():
    if "nc" not in _CACHE:
        _CACHE["nc"] = _build()
    return _CACHE["nc"]


def kernel(x, wq, bq, wk, bk, wv, bv, wp, bp, wbias):
    from concourse.bass_utils import run_bass_kernel_spmd

    x = np.asarray(x); wbias = np.asarray(wbias)
    wkv = np.concatenate([np.asarray(wv).T, np.asarray(wk).T], axis=1).astype(BF16)
    bkv = np.concatenate([np.asarray(bv), np.asarray(bk)])[None, :].astype(BF16)
    wqt = np.asarray(wq).T.astype(BF16)
    wpta = np.concatenate([np.asarray(wp).T, np.asarray(bp)[None, :]],
                          axis=0).astype(BF16)
    bq_in = (-np.asarray(bq)).reshape(H, 1).astype(np.float32)

    in_maps = []
    for c in range(NCORES):
        g, h = c % G, c // G
        tsl = slice(g * TSL, (g + 1) * TSL)
        in_maps.append({
            "xb": x[G * h + g].astype(BF16),
            "xq": x[NB * h:NB * (h + 1), :, tsl].astype(BF16),
            "ewb": np.ascontiguousarray(wbias[tsl, :].T).astype(BF16),
            "wkv": wkv, "wqt": wqt, "wpta": wpta, "bq": bq_in, "bkv": bkv,
        })

    nc = _get_nc()
    res = run_bass_kernel_spmd(nc, in_maps, core_ids=list(range(NCORES)),
                               **RUN_KWARGS)
    LAST_RESULT[0] = res

    out_full = np.empty((B, T, DIM), np.float32)
    for c in range(NCORES):
        g, h = c % G, c // G
        out_full[NB * h:NB * (h + 1), g * TSL:(g + 1) * TSL, :] = \
            res.results[c]["out"]
    return (out_full, out_full)
